# revision 5
# baseline (speedup 1.0000x reference)
"""Dynamic 3x3 per-pixel filter (DynamicFilterLayer2D) on 8 Trainium2 cores.

Reference: out[b,c,h,w] = sum_{i,j in 3x3} xpad[b,c,h+i,w+j] * f[b,c,(3i+j),h,w]

Sharding: H is split into 8 bands of 32 rows; each core processes all
(b, c) images for its band (data parallel, 1-row halo). Per-core layout:
partitions = 128 (b,c) images (2 groups of 128), free dim = flat pixels.

All HBM traffic is fp16 (the 2e-2 rel-err gate leaves ~20x margin), which
halves DMA bytes versus fp32; per-core DMA roofline is ~371 GB/s.

Compute is pure DVE in 2x fp16 mode (measured 0.59 ns/elem; GPSIMD
tensor ops share SBUF ports with DVE and slow it ~4x when co-active, so
they are not used). Filters are staged planar (9 per-tap planes): each
tap's product is a contiguous multiply where the 3x3 window shift is an
offset into the row-flat x tile; one fused multiply covers all 9 planes
via a [[W,3],[1,3],[1,FS]] read of x. The 9 planes are summed by a
4-instruction in-place binary tree. Filter border columns (taps that
would read x column padding) are zeroed host-side, so column wrap reads
multiply garbage by 0 and no x column padding is needed.
"""

import numpy as np

B, C, H, W = 8, 32, 256, 256
K = 3
N_CORES = 8
BAND = H // N_CORES            # 32 rows per core
RD = 8                         # rows per full super-tile
FS = RD * W                    # pixels per partition per super-tile (2048)
N_IMG = B * C                  # 256 images
P = 128
GROUPS = N_IMG // P            # 2
XLEN = (BAND + 2) * W + 2      # per-image padded x row storage (8706)

_CACHE = {}


def _strided_ap(tile_ap, dims, offset):
    """Copy of tile_ap with free dims replaced by [[step, count], ...]
    (element units) at element offset; partition dim preserved."""
    import bass_rust
    c = tile_ap.copy()
    part = list(c.ap)[0]
    c.ap = bass_rust.VecI64Pair([list(part)] + [list(d) for d in dims])
    c.offset = offset
    return c


def _build_module():
    import concourse.bacc as bacc
    import concourse.mybir as mybir
    from concourse.tile import TileContext

    fp16 = mybir.dt.float16
    add = mybir.AluOpType.add
    mult = mybir.AluOpType.mult

    nc = bacc.Bacc("TRN2", target_bir_lowering=False, debug=False)
    x_d = nc.dram_tensor("x_s", [N_IMG, XLEN], fp16,
                         kind="ExternalInput").ap()
    # planar taps: [img, tap, band_row, w]
    f_d = nc.dram_tensor("f_s", [N_IMG, K * K, BAND, W], fp16,
                         kind="ExternalInput").ap()
    o_d = nc.dram_tensor("o_s", [N_IMG, BAND, W], fp16,
                         kind="ExternalOutput").ap()

    # tiny leading supers shorten the initial f/x-DMA ramp before the
    # DVE can start; small trailing supers shorten the drain tail.
    # Small supers draw from their own f pool so prefetch of the big
    # supers is not blocked on buffer recycling.
    supers = {0: [(0, 2), (2, 2), (4, 4), (RD, RD), (2 * RD, RD),
                  (3 * RD, RD)],
              1: [(0, RD), (RD, RD), (2 * RD, RD), (3 * RD, RD // 2),
                  (3 * RD + RD // 2, RD // 2)]}

    with TileContext(nc) as tc:
        with (
            tc.tile_pool(name="xp", bufs=3) as xpool,
            tc.tile_pool(name="fb", bufs=2) as fbig,
            tc.tile_pool(name="fs", bufs=2) as fsmall,
            tc.tile_pool(name="pp", bufs=1) as ppool,
            tc.tile_pool(name="p3", bufs=2) as p3pool,
            tc.tile_pool(name="op", bufs=2) as opool,
        ):
            for g in range(GROUPS):
                p0 = g * P
                for (r0, rd) in supers[g]:
                    fs = rd * W
                    xlen = (rd + 2) * W + 2
                    fpool = fsmall if rd <= 2 else fbig
                    ft = fpool.tile([P, K * K * W * (2 if rd <= 2 else RD)],
                                    fp16, tag="f")
                    nc.sync.dma_start(
                        out=ft[:, 0:K * K * fs],
                        in_=f_d[p0:p0 + P, :, r0:r0 + rd, :],
                    )
                    xt = xpool.tile([P, (RD + 2) * W + 2], fp16, tag="x")
                    nc.scalar.dma_start(
                        out=xt[:, 0:xlen],
                        in_=x_d[p0:p0 + P, r0 * W:r0 * W + xlen],
                    )
                    # products: taps 0-5 (rows i=0,1) planar into pt;
                    # taps 6-8 (row i=2) into p3 (read by the accum DMA)
                    pt = ppool.tile([P, 6 * FS], fp16, tag="p")
                    p3 = p3pool.tile([P, 3 * FS], fp16, tag="q")
                    xinA = _strided_ap(xt[:, :], [[W, 2], [1, K], [1, fs]], 0)
                    finA = _strided_ap(ft[:, :], [[K * fs, 2], [fs, K],
                                                  [1, fs]], 0)
                    poutA = _strided_ap(pt[:, :], [[K * fs, 2], [fs, K],
                                                   [1, fs]], 0)
                    nc.vector.tensor_tensor(poutA, xinA, finA, mult)
                    xinB = _strided_ap(xt[:, :], [[1, K], [1, fs]], 2 * W)
                    finB = _strided_ap(ft[:, :], [[fs, K], [1, fs]],
                                       2 * K * fs)
                    poutB = _strided_ap(p3[:, :], [[fs, K], [1, fs]], 0)
                    nc.vector.tensor_tensor(poutB, xinB, finB, mult)
                    # sum: DVE does 8 of the 9 planes; the 9th is folded in
                    # by a gpsimd accumulating SBUF->SBUF DMA
                    nc.vector.tensor_tensor(
                        pt[:, 0:3 * fs], pt[:, 0:3 * fs],
                        pt[:, 3 * fs:6 * fs], add)
                    nc.vector.tensor_tensor(
                        pt[:, 0:fs], pt[:, 0:fs], pt[:, fs:2 * fs], add)
                    nc.vector.tensor_tensor(
                        pt[:, 0:fs], pt[:, 0:fs], pt[:, 2 * fs:3 * fs], add)
                    nc.vector.tensor_tensor(
                        p3[:, 0:fs], p3[:, 0:fs], p3[:, fs:2 * fs], add)
                    ot = opool.tile([P, FS], fp16, tag="o")
                    nc.vector.tensor_tensor(
                        ot[:, 0:fs], pt[:, 0:fs], p3[:, 0:fs], add)
                    nc.gpsimd.dma_start(
                        out=ot[:, 0:fs], in_=p3[:, 2 * fs:3 * fs],
                        accum_op=add)
                    nc.scalar.dma_start(
                        out=o_d[p0:p0 + P, r0:r0 + rd, :],
                        in_=ot[:, 0:fs],
                    )
    nc.compile()
    return nc


def _get_module():
    if "nc" not in _CACHE:
        _CACHE["nc"] = _build_module()
    return _CACHE["nc"]


def _shard_inputs(x, dynamic_filters):
    """Per-core input maps. x: [B,C,H,W] f32, filters: [B,C*9,H,W] f32."""
    xp = np.pad(x, ((0, 0), (0, 0), (1, 1), (0, 0))).astype(np.float16)
    # planar taps [img, t, H, W]; zero border cols (j=0 @ w=0, j=2 @ w=W-1)
    fp = np.ascontiguousarray(
        dynamic_filters.reshape(N_IMG, K * K, H, W)).astype(np.float16)
    fp[:, 0::3, :, 0] = 0.0
    fp[:, 2::3, :, W - 1] = 0.0

    in_maps = []
    for n in range(N_CORES):
        r = n * BAND
        xs = xp[:, :, r:r + BAND + 2, :].reshape(N_IMG, (BAND + 2) * W)
        xs_flat = np.zeros((N_IMG, XLEN), np.float16)
        xs_flat[:, 1:-1] = xs
        fs = np.ascontiguousarray(fp[:, :, r:r + BAND])
        in_maps.append({"x_s": xs_flat, "f_s": fs})
    return in_maps


def kernel(x, dynamic_filters, _trace=False):
    from concourse import bass_utils

    x = np.asarray(x, dtype=np.float32)
    dynamic_filters = np.asarray(dynamic_filters, dtype=np.float32)
    nc = _get_module()
    in_maps = _shard_inputs(x, dynamic_filters)
    res = bass_utils.run_bass_kernel_spmd(
        nc, in_maps, list(range(N_CORES)), trace=_trace)
    out = np.concatenate(
        [res.results[n]["o_s"].reshape(B, C, BAND, W).astype(np.float32)
         for n in range(N_CORES)],
        axis=2)
    _CACHE["last_exec_time_ns"] = res.exec_time_ns
    return out


# revision 7
# speedup vs baseline: 1.2643x; 1.2643x over previous
"""Dynamic 3x3 per-pixel filter (DynamicFilterLayer2D) on 8 Trainium2 cores.

Reference: out[b,c,h,w] = sum_{i,j in 3x3} xpad[b,c,h+i,w+j] * f[b,c,(3i+j),h,w]

Sharding: H is split into 8 bands of 32 rows; each core processes all
(b, c) images for its band (data parallel, 1-row halo). Per-core layout:
partitions = 128 (b,c) images (2 groups of 128), free dim = flat pixels.

All HBM traffic is fp16 (the 2e-2 rel-err gate leaves ~20x margin), which
halves DMA bytes versus fp32; per-core DMA roofline is ~371 GB/s.

Engine split (rates measured on HW):
- DVE in 2x fp16 mode (0.59 ns/elem): two fused multiplies produce the 9
  per-tap product planes (the 3x3 window shift is just an offset into the
  row-flat x tile, so every AP is packed-contiguous and 2x-eligible),
  then 3 adds fold planes 0..5 into one partial.
- PE sums planes 6..8 plus that partial into PSUM via identity-weight
  matmuls (PSUM accumulation) - 4 chunked matmuls per 512-col PSUM bank.
- ACT copies PSUM (fp32) to the fp16 out tile and issues the out DMA.
GPSIMD tensor ops share SBUF ports with DVE (measured ~4x slowdown when
co-active) and are not used.

Filter border columns (taps that would read x column padding) are zeroed
host-side, so column wrap reads multiply garbage by 0 and no x column
padding is needed.
"""

import numpy as np

B, C, H, W = 8, 32, 256, 256
K = 3
N_CORES = 8
BAND = H // N_CORES            # 32 rows per core
RD = 8                         # rows per full super-tile
FS = RD * W                    # pixels per partition per super-tile (2048)
N_IMG = B * C                  # 256 images
P = 128
GROUPS = N_IMG // P            # 2
XLEN = (BAND + 2) * W + 2      # per-image padded x row storage (8706)
PSUM_CHUNK = 512               # fp32 elems per PSUM bank per partition

_CACHE = {}


def _strided_ap(tile_ap, dims, offset):
    """Copy of tile_ap with free dims replaced by [[step, count], ...]
    (element units) at element offset; partition dim preserved."""
    import bass_rust
    c = tile_ap.copy()
    part = list(c.ap)[0]
    c.ap = bass_rust.VecI64Pair([list(part)] + [list(d) for d in dims])
    c.offset = offset
    return c


def _build_module():
    import concourse.bacc as bacc
    import concourse.mybir as mybir
    from concourse.tile import TileContext

    fp16 = mybir.dt.float16
    fp32 = mybir.dt.float32
    add = mybir.AluOpType.add
    mult = mybir.AluOpType.mult

    nc = bacc.Bacc("TRN2", target_bir_lowering=False, debug=False)
    x_d = nc.dram_tensor("x_s", [N_IMG, XLEN], fp16,
                         kind="ExternalInput").ap()
    # planar taps: [img, tap, band_row, w]
    f_d = nc.dram_tensor("f_s", [N_IMG, K * K, BAND, W], fp16,
                         kind="ExternalInput").ap()
    eye_d = nc.dram_tensor("eye_s", [P, P], fp16, kind="ExternalInput").ap()
    o_d = nc.dram_tensor("o_s", [N_IMG, BAND, W], fp16,
                         kind="ExternalOutput").ap()

    # tiny leading supers shorten the initial f/x-DMA ramp before the
    # DVE can start; small trailing supers shorten the drain tail.
    # Small supers draw from their own f pool so prefetch of the big
    # supers is not blocked on buffer recycling.
    supers = {0: [(0, 2), (2, 2), (4, 4), (RD, RD), (2 * RD, RD),
                  (3 * RD, RD)],
              1: [(0, RD), (RD, RD), (2 * RD, RD), (3 * RD, RD // 2),
                  (3 * RD + RD // 2, RD // 2)]}

    with TileContext(nc) as tc:
        with (
            tc.tile_pool(name="ey", bufs=1) as epool,
            tc.tile_pool(name="xp", bufs=3) as xpool,
            tc.tile_pool(name="fb", bufs=2) as fbig,
            tc.tile_pool(name="fs", bufs=2) as fsmall,
            tc.tile_pool(name="pp", bufs=1) as ppool,
            tc.tile_pool(name="p3", bufs=2) as p3pool,
            tc.tile_pool(name="st", bufs=2) as spool,
            tc.tile_pool(name="op", bufs=2) as opool,
            tc.tile_pool(name="ps", bufs=2, space="PSUM") as psumpool,
        ):
            eye = epool.tile([P, P], fp16, tag="eye")
            nc.sync.dma_start(out=eye[:, :], in_=eye_d[:, :])
            for g in range(GROUPS):
                p0 = g * P
                for (r0, rd) in supers[g]:
                    fs = rd * W
                    xlen = (rd + 2) * W + 2
                    fpool = fsmall if rd <= 2 else fbig
                    ft = fpool.tile([P, K * K * W * (2 if rd <= 2 else RD)],
                                    fp16, tag="f")
                    nc.sync.dma_start(
                        out=ft[:, 0:K * K * fs],
                        in_=f_d[p0:p0 + P, :, r0:r0 + rd, :],
                    )
                    xt = xpool.tile([P, (RD + 2) * W + 2], fp16, tag="x")
                    nc.scalar.dma_start(
                        out=xt[:, 0:xlen],
                        in_=x_d[p0:p0 + P, r0 * W:r0 * W + xlen],
                    )
                    # products: taps 0-5 (rows i=0,1) planar into pt;
                    # taps 6-8 (row i=2) into p3 (read by PE)
                    pt = ppool.tile([P, 6 * FS], fp16, tag="p")
                    p3 = p3pool.tile([P, 3 * FS], fp16, tag="q")
                    xinA = _strided_ap(xt[:, :], [[W, 2], [1, K], [1, fs]], 0)
                    finA = _strided_ap(ft[:, :], [[K * fs, 2], [fs, K],
                                                  [1, fs]], 0)
                    poutA = _strided_ap(pt[:, :], [[K * fs, 2], [fs, K],
                                                   [1, fs]], 0)
                    nc.vector.tensor_tensor(poutA, xinA, finA, mult)
                    xinB = _strided_ap(xt[:, :], [[1, K], [1, fs]], 2 * W)
                    finB = _strided_ap(ft[:, :], [[fs, K], [1, fs]],
                                       2 * K * fs)
                    poutB = _strided_ap(p3[:, :], [[fs, K], [1, fs]], 0)
                    nc.vector.tensor_tensor(poutB, xinB, finB, mult)
                    # DVE folds planes 0..5 into st
                    nc.vector.tensor_tensor(
                        pt[:, 0:3 * fs], pt[:, 0:3 * fs],
                        pt[:, 3 * fs:6 * fs], add)
                    nc.vector.tensor_tensor(
                        pt[:, 0:fs], pt[:, 0:fs], pt[:, fs:2 * fs], add)
                    st = spool.tile([P, FS], fp16, tag="s")
                    nc.vector.tensor_tensor(
                        st[:, 0:fs], pt[:, 0:fs], pt[:, 2 * fs:3 * fs], add)
                    # PE: psum = p6 + p7 + p8 + st, one bank per 512 cols
                    pst = psumpool.tile([P, FS], fp32, tag="ps")
                    nch = (fs + PSUM_CHUNK - 1) // PSUM_CHUNK
                    for c in range(nch):
                        lo = c * PSUM_CHUNK
                        hi = min(fs, lo + PSUM_CHUNK)
                        for k, src in enumerate((p3[:, lo:hi],
                                                 p3[:, fs + lo:fs + hi],
                                                 p3[:, 2 * fs + lo:2 * fs + hi],
                                                 st[:, lo:hi])):
                            nc.tensor.matmul(
                                out=pst[:, lo:hi], lhsT=eye[:, :], rhs=src,
                                start=(k == 0), stop=(k == 3),
                            )
                    # ACT: downcast PSUM to the fp16 out tile
                    ot = opool.tile([P, FS], fp16, tag="o")
                    nc.scalar.copy(ot[:, 0:fs], pst[:, 0:fs])
                    nc.scalar.dma_start(
                        out=o_d[p0:p0 + P, r0:r0 + rd, :],
                        in_=ot[:, 0:fs],
                    )
    nc.compile()
    return nc


def _get_module():
    if "nc" not in _CACHE:
        _CACHE["nc"] = _build_module()
    return _CACHE["nc"]


def _shard_inputs(x, dynamic_filters):
    """Per-core input maps. x: [B,C,H,W] f32, filters: [B,C*9,H,W] f32."""
    xp = np.pad(x, ((0, 0), (0, 0), (1, 1), (0, 0))).astype(np.float16)
    # planar taps [img, t, H, W]; zero border cols (j=0 @ w=0, j=2 @ w=W-1)
    fp = np.ascontiguousarray(
        dynamic_filters.reshape(N_IMG, K * K, H, W)).astype(np.float16)
    fp[:, 0::3, :, 0] = 0.0
    fp[:, 2::3, :, W - 1] = 0.0
    eye = np.eye(P, dtype=np.float16)

    in_maps = []
    for n in range(N_CORES):
        r = n * BAND
        xs = xp[:, :, r:r + BAND + 2, :].reshape(N_IMG, (BAND + 2) * W)
        xs_flat = np.zeros((N_IMG, XLEN), np.float16)
        xs_flat[:, 1:-1] = xs
        fs = np.ascontiguousarray(fp[:, :, r:r + BAND])
        in_maps.append({"x_s": xs_flat, "f_s": fs, "eye_s": eye})
    return in_maps


def kernel(x, dynamic_filters, _trace=False):
    from concourse import bass_utils

    x = np.asarray(x, dtype=np.float32)
    dynamic_filters = np.asarray(dynamic_filters, dtype=np.float32)
    nc = _get_module()
    in_maps = _shard_inputs(x, dynamic_filters)
    res = bass_utils.run_bass_kernel_spmd(
        nc, in_maps, list(range(N_CORES)), trace=_trace)
    out = np.concatenate(
        [res.results[n]["o_s"].reshape(B, C, BAND, W).astype(np.float32)
         for n in range(N_CORES)],
        axis=2)
    _CACHE["last_exec_time_ns"] = res.exec_time_ns
    return out
